# revision 1
# baseline (speedup 1.0000x reference)
"""Dilated attention (LongNet-style) Trainium2 kernel.

Problem: query/key/value (2, 8192, 12, 64) f32. Three dilation groups
(segment lengths 2048/4096/8192, dilation 1/2/4, head slices 0:4/4:8/8:12).
Each group's gather produces independent dense attention over 2048-position
dilated segments; outputs are normalized per (batch, head, channel) by the
sum over all segment positions, and divided by num_groups.

Sharding: 8 cores = 2 batches x 4 "head columns". Core c owns batch c//4 and
heads {j, 4+j, 8+j} where j = c%4 -- exactly 7 dense 2048x2048x64 attention
units per core (4 + 2 + 1 segments), perfectly balanced, with all segments of
any (batch, head) on one core so normalization needs no cross-core traffic.

Precision: the reference's x / x.sum(axis=(1,2)) normalization divides by a
nearly-cancelling sum, which amplifies independent per-element error ~300x.
bf16 matmuls (rel ~0.6) and even float32r (rel ~0.12) fail; the kernel needs
~fp32 effective precision, built from fp16 hi/lo pairs (~22 bit):
 - scores = (qh+ql)@(kh+kl)^T with fp16 pairs of 256*q (pre-scale keeps
   ql/kl out of the fp16 subnormal range; the 2^-16 descale folds into the
   exp scale): 2 PE cycles/row with full K=128 stacking.
 - P = 64*exp(score) computed f32 in-place in PSUM by ACT (the *64 comes
   from bias=ln 64 and lifts fp16(P) fully into normal range; it cancels in
   the final num/den ratio), then split by DVE into fp16 p1 + p2.
 - V pre-scaled by 256 and split into fp16 v1h + v1l on the host (also
   cancels in num/den).
 - P@V = p1@v1h + p2@v1h + p1@v1l, 3 full-rate fp16 matmuls per k-block
   accumulating in f32 PSUM (the dropped p2*v1l term is ~2^-22 relative).
End-to-end measured: 510 us HW exec on 8 cores (PE 95% busy, every
matmul at the full 216 ns N=512 stream rate; algorithmic PE floor is
5 streams/unit = 478 us), rel err 1.5e-4 vs a strict-fp32 CPU reference
(honest-fp32 baseline envelope is ~1.7e-5).

Device kernel (same program on all 8 cores, different data):
  - inputs (per segment s, d on partitions for Q/K):
      qhh [128, 14336] fp16: rows 0-63 = qh = fp16(256*Q^T), rows 64-127 dup
      qll [128, 14336] fp16: ql = fp16(256*Q^T - qh), duplicated rows
      khl [128, 14336] fp16: rows 0-63 = kh, rows 64-127 = kl
      v1h/v1l [128, 7280] fp16 pair: 256*V per 128-row k-block + ones
      column = 256 in v1h (softmax denominator)
  - per (chunk, k-block) unit (28 q-chunks of 512 x 16 k-blocks):
      S^T = khl_blk.T @ qhh + khl_blk.T @ qll   (PE, 1 LDW + 2 MMs, PSUM f32)
      P = exp(S^T*0.125/65536 + ln64) in-place  (ACT, 3-bank spans)
      p1 = fp16(P); p2 = fp16(P - p1)           (DVE, PSUM -> SBUF)
      O'[65, 512] += v1h.T@p1 + v1h.T@p2 + v1l.T@p1   (PE fp16, f32 PSUM,
                                accumulated over kb; row 64 = denominator)
  - O' copied PSUM->SBUF (DVE) and streamed to DRAM out [65, 14336] f32.
Host divides by the denominator row, applies the group normalization
(sum over positions per channel) and the /3, and scatters into the full
(2, 8192, 12, 64) output. Positions not in a dilated group stay zero.
"""

import os
import sys

if "/opt/trn_rl_repo" not in sys.path:
    sys.path.insert(0, "/opt/trn_rl_repo")
if "jax" not in sys.modules:
    os.environ.setdefault("JAX_PLATFORMS", "axon")

import numpy as np

import concourse.bass as bass  # noqa: F401
import concourse.mybir as mybir
import concourse.tile as tile
from concourse import bacc
from concourse.bass_utils import run_bass_kernel_spmd

F32 = mybir.dt.float32
F16 = mybir.dt.float16

B, N, H, D = 2, 8192, 12, 64
NSEG = 7           # segments per core
SEG = 2048         # dilated segment length
NCHUNK = NSEG * 4  # 512-wide q chunks per core
NKB = 16           # 128-row k blocks per segment
NUNIT = NCHUNK * NKB
RW = 3             # k-blocks per exp round (3 PSUM banks per ACT span)
QSC = np.float32(256.0)               # fp16 pre-scale for Q/K/V splits
ESC = float(0.125 / (256.0 * 256.0))  # exp scale: 1/sqrt(64) + descale
import math
PBIAS = float(math.log(64.0))         # exp bias: P *= 64, into fp16-normal range

_CACHE = {}
LAST_RESULT = {}


def _build_nc():
    nc = bacc.Bacc("TRN2", target_bir_lowering=False, debug=False,
                   enable_asserts=False, num_devices=8)
    qhh = nc.dram_tensor("qhh", [128, NSEG * SEG], F16, kind="ExternalInput")
    qll = nc.dram_tensor("qll", [128, NSEG * SEG], F16, kind="ExternalInput")
    khl = nc.dram_tensor("khl", [128, NSEG * SEG], F16, kind="ExternalInput")
    v1h = nc.dram_tensor("v1h", [128, NSEG * NKB * 65], F16, kind="ExternalInput")
    v1l = nc.dram_tensor("v1l", [128, NSEG * NKB * 65], F16, kind="ExternalInput")
    out = nc.dram_tensor("out", [65, NCHUNK * 512], F32, kind="ExternalOutput")
    qhh_ap, qll_ap, khl_ap, v1h_ap, v1l_ap, out_ap = (
        qhh.ap(), qll.ap(), khl.ap(), v1h.ap(), v1l.ap(), out.ap())

    with tile.TileContext(nc) as tc:
        with (
            tc.tile_pool(name="inp", bufs=1) as inp,
            tc.tile_pool(name="pt", bufs=5) as ptp,
            tc.tile_pool(name="osb", bufs=3) as osbp,
            tc.tile_pool(name="score", bufs=2, space="PSUM") as scp,
            tc.tile_pool(name="ot", bufs=2, space="PSUM") as otp,
        ):
            bias_t = inp.tile([128, 1], F32, tag="bias", name="bias_t")
            nc.vector.memset(bias_t[:, :], PBIAS)

            # Warm-up prologue: runs while the input DMAs land. ~24 dummy
            # matmuls keep the PE busy >3.4us so the HAM clock-gate opens
            # before the real rounds, and one dummy exp pulls in the ACT
            # table load (~2.7us) that would otherwise stall round 0.
            wsrc = inp.tile([128, 128], F16, tag="wsrc", name="wsrc")
            wjunk = inp.tile([128, 512], F16, tag="wjunk", name="wjunk")
            nc.vector.memset(wsrc[:, :], 0.01)
            nc.vector.memset(wjunk[:, :], 0.01)
            warm = scp.tile([128, 512 * RW], F32, tag="score", name="warm")
            for i in range(32):
                nc.tensor.matmul(warm[:, (i % 3) * 512:(i % 3 + 1) * 512],
                                 wsrc[:, :], wjunk[:, :],
                                 start=(i < 3), stop=(i >= 29))
            wp = ptp.tile([128, 512 * RW], F16, tag="p1", name="warmp")
            nc.scalar.activation(
                wp[:, :512], warm[:, :512],
                mybir.ActivationFunctionType.Exp, scale=ESC, bias=bias_t[:, :])

            qh_sb, ql_sb, k_sb, vh_sb, vl_sb = [], [], [], [], []
            for s in range(NSEG):
                qh = inp.tile([128, SEG], F16, tag=f"qh{s}", name=f"qh{s}")
                ql = inp.tile([128, SEG], F16, tag=f"ql{s}", name=f"ql{s}")
                kk = inp.tile([128, SEG], F16, tag=f"k{s}", name=f"k{s}")
                vh = inp.tile([128, NKB * 65], F16, tag=f"vh{s}", name=f"vh{s}")
                vl = inp.tile([128, NKB * 65], F16, tag=f"vl{s}", name=f"vl{s}")
                vsl = slice(s * NKB * 65, (s + 1) * NKB * 65)
                # split the first segment's Q/K transfers across DMA queues so
                # round 0 isn't gated on a single ~512KB queue transfer
                nsl_dma = 4 if s == 0 else 1
                for t, ap_ in ((qh, qhh_ap), (ql, qll_ap), (kk, khl_ap)):
                    step = SEG // nsl_dma
                    for z in range(nsl_dma):
                        lo = z * step
                        nc.sync.dma_start(
                            t[:, lo:lo + step],
                            ap_[:, s * SEG + lo:s * SEG + lo + step])
                nc.sync.dma_start(vh[:, :], v1h_ap[:, vsl])
                nc.sync.dma_start(vl[:, :], v1l_ap[:, vsl])
                qh_sb.append(qh)
                ql_sb.append(ql)
                k_sb.append(kk)
                vh_sb.append(vh)
                vl_sb.append(vl)

            ot_tiles = {}
            pend1, pend2 = [], []  # PV work lagged by 1 and 2 rounds

            def flush(items):
                for p1ref, p2ref, i, u in items:
                    cid, kb = divmod(u, NKB)
                    s = cid // 4
                    if kb == 0:
                        ot_tiles[cid] = otp.tile([65, 512], F32, tag="ot",
                                                 name=f"ot{cid}")
                    vsl = slice(kb * 65, (kb + 1) * 65)
                    psl = slice(i * 512, (i + 1) * 512)
                    ot = ot_tiles[cid][:, :]
                    nc.tensor.matmul(ot, vh_sb[s][:, vsl], p1ref[:, psl],
                                     start=(kb == 0), stop=False)
                    nc.tensor.matmul(ot, vh_sb[s][:, vsl], p2ref[:, psl],
                                     start=False, stop=False)
                    nc.tensor.matmul(ot, vl_sb[s][:, vsl], p1ref[:, psl],
                                     start=False, stop=(kb == NKB - 1))
                    if kb == NKB - 1:
                        o_sb = osbp.tile([65, 512], F32, tag="osb",
                                         name=f"osb{cid}")
                        nc.vector.tensor_copy(o_sb[:, :], ot_tiles[cid][:, :])
                        nc.sync.dma_start(
                            out_ap[:, cid * 512:(cid + 1) * 512], o_sb[:, :])

            for r in range((NUNIT + RW - 1) // RW):
                units = range(r * RW, min((r + 1) * RW, NUNIT))
                nu = len(units)
                score = scp.tile([128, 512 * RW], F32, tag="score",
                                 name=f"score{r}")
                for i, u in enumerate(units):
                    cid, kb = divmod(u, NKB)
                    s, c = divmod(cid, 4)
                    osl = slice(i * 512, (i + 1) * 512)
                    csl = slice(c * 512, (c + 1) * 512)
                    lhsT = k_sb[s][:, kb * 128:(kb + 1) * 128]
                    nc.tensor.matmul(score[:, osl], lhsT, qh_sb[s][:, csl],
                                     start=True, stop=False)
                    nc.tensor.matmul(score[:, osl], lhsT, ql_sb[s][:, csl],
                                     start=False, stop=True)
                nsl = slice(0, 512 * nu)
                p1 = ptp.tile([128, 512 * RW], F16, tag="p1", name=f"p1_{r}")
                p2 = ptp.tile([128, 512 * RW], F16, tag="p2", name=f"p2_{r}")
                # p1 = fp16(64*exp(s)) straight from ACT; then the same exp
                # in-place f32 (identical spline -> identical value), and the
                # fp16 residual on DVE.
                nc.scalar.activation(
                    p1[:, nsl], score[:, nsl],
                    mybir.ActivationFunctionType.Exp, scale=ESC,
                    bias=bias_t[:, :])
                nc.scalar.activation(
                    score[:, nsl], score[:, nsl],
                    mybir.ActivationFunctionType.Exp, scale=ESC,
                    bias=bias_t[:, :])
                nc.vector.tensor_sub(p2[:, nsl], score[:, nsl], p1[:, nsl])
                if r < 2:
                    # startup filler: the first PV work arrives only after the
                    # round-0 scores->exp->exp->sub chain (~5us); keep the PE
                    # streaming through the pipe-fill with dummies aimed at an
                    # OT-pool slot (idle until round 2).
                    fill = otp.tile([128, 512], F32, tag="ot", name=f"fill{r}")
                    for z in range(7):
                        nc.tensor.matmul(fill[:, :], wsrc[:, :], wjunk[:, :],
                                         start=(z == 0), stop=(z == 6))
                flush(pend2)
                pend2 = pend1
                pend1 = [(p1, p2, i, u) for i, u in enumerate(units)]
            flush(pend2)
            flush(pend1)

    nc.compile()
    return nc


def _prep_core(query, key, value, core):
    b, j = divmod(core, 4)
    segs = []
    for arr in (query, key, value):
        h0 = arr[b, :, j, :].reshape(4, SEG, D)
        h1 = arr[b, :, 4 + j, :].reshape(2, 4096, D)[:, 1::2, :]
        h2 = arr[b, 2::4, 8 + j, :][None]
        segs.append(np.concatenate([h0, h1, h2], axis=0))  # [7, 2048, 64]
    qs, ks, vs = segs
    # [64, NSEG*SEG] with col = s*SEG + p
    qt = (qs * QSC).transpose(2, 0, 1).reshape(D, NSEG * SEG)
    kt = (ks * QSC).transpose(2, 0, 1).reshape(D, NSEG * SEG)
    qh = qt.astype(np.float16)
    ql = (qt - qh).astype(np.float16)
    kh = kt.astype(np.float16)
    kl = (kt - kh).astype(np.float16)
    vv = np.concatenate(
        [vs * QSC, np.full((NSEG, SEG, 1), 256.0, np.float32)],
        axis=2)  # [7, 2048, 65], pre-scaled
    v1 = vv.reshape(NSEG, NKB, 128, 65).transpose(2, 0, 1, 3).reshape(128, -1)
    v1h = v1.astype(np.float16)
    v1l = (v1 - v1h).astype(np.float16)
    return {
        "qhh": np.ascontiguousarray(np.concatenate([qh, qh], axis=0)),
        "qll": np.ascontiguousarray(np.concatenate([ql, ql], axis=0)),
        "khl": np.ascontiguousarray(np.concatenate([kh, kl], axis=0)),
        "v1h": np.ascontiguousarray(v1h),
        "v1l": np.ascontiguousarray(v1l),
    }


def _unshard(results, dtype):
    full = np.zeros((B, N, H, D), dtype)
    for core in range(8):
        b, j = divmod(core, 4)
        o = results[core]["out"].astype(np.float64)
        T = o[:64] / o[64:65]  # [64, 14336]
        h0 = T[:, :4 * SEG]
        full[b, :, j, :] = (h0 / (3.0 * h0.sum(1, keepdims=True))).T
        h1 = T[:, 4 * SEG:6 * SEG]
        h1 = h1 / (3.0 * h1.sum(1, keepdims=True))
        for g in range(2):
            full[b, g * 4096 + 1:(g + 1) * 4096:2, 4 + j, :] = \
                h1[:, g * SEG:(g + 1) * SEG].T
        h2 = T[:, 6 * SEG:]
        full[b, 2::4, 8 + j, :] = (h2 / (3.0 * h2.sum(1, keepdims=True))).T
    return full


def _ensure_axon_backend():
    """The bass PJRT path needs the axon/neuron jax backend. A harness may
    pin JAX_PLATFORMS=cpu for its reference; re-select axon if so."""
    import jax
    try:
        plat = jax.devices()[0].platform
    except Exception:
        plat = ""
    if plat not in ("axon", "neuron"):
        try:
            jax.config.update("jax_platforms", "axon,cpu")
            jax.devices()
        except Exception:
            pass


def kernel(query, key, value):
    _ensure_axon_backend()
    query = np.asarray(query, np.float32)
    key = np.asarray(key, np.float32)
    value = np.asarray(value, np.float32)
    assert query.shape == (B, N, H, D)

    if "nc" not in _CACHE:
        _CACHE["nc"] = _build_nc()
    nc = _CACHE["nc"]

    in_maps = [_prep_core(query, key, value, c) for c in range(8)]
    res = run_bass_kernel_spmd(nc, in_maps, core_ids=list(range(8)))
    LAST_RESULT["exec_time_ns"] = res.exec_time_ns
    return _unshard(res.results, query.dtype)



# revision 2
# speedup vs baseline: 1.6132x; 1.6132x over previous
"""Dilated attention (LongNet-style) Trainium2 kernel — v2.

Problem: query/key/value (2, 8192, 12, 64) f32. Three dilation groups
(segment lengths 2048/4096/8192, dilation 1/2/4, head slices 0:4/4:8/8:12).
Each group's gather produces independent dense attention over 2048-position
dilated segments; outputs are normalized per (batch, head, channel) by the
sum over all segment positions, and divided by num_groups.

Sharding: 8 cores = 2 batches x 4 "head columns". Core c owns batch c//4 and
heads {j, 4+j, 8+j} where j = c%4 -- exactly 7 dense 2048x2048x64 attention
units per core (4 + 2 + 1 segments), perfectly balanced, with all segments of
any (batch, head) on one core so normalization needs no cross-core traffic.

Precision (validated by numpy simulation of the exact arithmetic): the
x / x.sum normalization amplifies V-path errors ~140x but score/P-path
errors only ~8-15x. So:
 - K is kept to ~fp32 via an fp16 hi/lo pair packed along the contraction
   dim (khl rows 0-63 = kh, 64-127 = kl; qhh rows = qh duplicated), giving
   scores = (kh+kl)^T qh in ONE fp16 matmul. Q single-fp16 is enough
   (sim: +1e-3).
 - P = 64*exp(score) rounded to fp16 directly by ACT (single pass, no
   residual pair needed; sim: +2e-3).
 - V must stay an fp16 hi/lo pair: PV = p1@vh + p1@vl (dropping vl fails
   at 2.4e-2). Softmax denominator rides as the ones column of vh.
Total: 3 matmul streams per 128x512 unit (vs 5 in v1), one ACT exp pass
(vs 2 + DVE sub). Sim end-to-end: 3.0e-3 (threshold 2e-2).

Device kernel (same program on all 8 cores, different data):
  - inputs (per segment s, d on partitions for Q/K):
      qhh [128, 14336] fp16: rows 0-63 = qh = fp16(256*Q^T), rows 64-127 dup
      khl [128, 14336] fp16: rows 0-63 = kh, rows 64-127 = kl
      v1h/v1l [128, 7280] fp16 pair: 256*V per 128-row k-block + ones
      column = 256 in v1h (softmax denominator)
  - per (chunk, k-block) unit (28 q-chunks of 512 x 16 k-blocks):
      S^T = khl_blk.T @ qhh                     (PE, 1 LDW + 1 MM, PSUM f32)
      p1 = fp16(exp(S^T*0.125/65536 + ln64))    (ACT, PSUM -> SBUF)
      O'[65, 512] += v1h.T@p1 + v1l.T@p1        (PE fp16, f32 PSUM,
                                accumulated over kb; row 64 = denominator)
  - O' copied PSUM->SBUF (DVE) and streamed to DRAM out [65, 14336] f32.
Host divides by the denominator row, applies the group normalization
(sum over positions per channel) and the /3, and scatters into the full
(2, 8192, 12, 64) output. Positions not in a dilated group stay zero.
"""

import os
import sys

if "/opt/trn_rl_repo" not in sys.path:
    sys.path.insert(0, "/opt/trn_rl_repo")
if "jax" not in sys.modules:
    os.environ.setdefault("JAX_PLATFORMS", "axon")

import numpy as np

import concourse.bass as bass  # noqa: F401
import concourse.mybir as mybir
import concourse.tile as tile
from concourse import bacc
from concourse.bass_utils import run_bass_kernel_spmd

F32 = mybir.dt.float32
F16 = mybir.dt.float16

B, N, H, D = 2, 8192, 12, 64
NSEG = 7           # segments per core
SEG = 2048         # dilated segment length
NCHUNK = NSEG * 4  # 512-wide q chunks per core
NKB = 16           # 128-row k blocks per segment
NUNIT = NCHUNK * NKB
RW = 3             # k-blocks per exp round (3 PSUM banks per ACT span)
QSC = np.float32(256.0)               # fp16 pre-scale for Q/K/V splits
ESC = float(0.125 / (256.0 * 256.0))  # exp scale: 1/sqrt(64) + descale
import math
PBIAS = float(math.log(64.0))         # exp bias: P *= 64, into fp16-normal range

_CACHE = {}
LAST_RESULT = {}


def _build_nc():
    nc = bacc.Bacc("TRN2", target_bir_lowering=False, debug=False,
                   enable_asserts=False, num_devices=8)
    qhh = nc.dram_tensor("qhh", [128, NSEG * SEG], F16, kind="ExternalInput")
    khl = nc.dram_tensor("khl", [128, NSEG * SEG], F16, kind="ExternalInput")
    v1h = nc.dram_tensor("v1h", [128, NSEG * NKB * 65], F16, kind="ExternalInput")
    v1l = nc.dram_tensor("v1l", [128, NSEG * NKB * 65], F16, kind="ExternalInput")
    out = nc.dram_tensor("out", [65, NCHUNK * 512], F32, kind="ExternalOutput")
    qhh_ap, khl_ap, v1h_ap, v1l_ap, out_ap = (
        qhh.ap(), khl.ap(), v1h.ap(), v1l.ap(), out.ap())

    with tile.TileContext(nc) as tc:
        with (
            tc.tile_pool(name="inp", bufs=1) as inp,
            tc.tile_pool(name="pt", bufs=4) as ptp,
            tc.tile_pool(name="osb", bufs=3) as osbp,
            tc.tile_pool(name="score", bufs=2, space="PSUM") as scp,
            tc.tile_pool(name="ot", bufs=2, space="PSUM") as otp,
        ):
            bias_t = inp.tile([128, 1], F32, tag="bias", name="bias_t")
            nc.vector.memset(bias_t[:, :], PBIAS)

            # Warm-up prologue: runs while the input DMAs land. ~32 dummy
            # matmuls keep the PE busy >3.4us so the HAM clock-gate opens
            # before the real rounds, and one dummy exp pulls in the ACT
            # table load (~2.7us) that would otherwise stall round 0.
            wsrc = inp.tile([128, 128], F16, tag="wsrc", name="wsrc")
            wjunk = inp.tile([128, 512], F16, tag="wjunk", name="wjunk")
            nc.vector.memset(wsrc[:, :], 0.01)
            nc.vector.memset(wjunk[:, :], 0.01)
            warm = scp.tile([128, 512 * RW], F32, tag="score", name="warm")
            for i in range(32):
                nc.tensor.matmul(warm[:, (i % 3) * 512:(i % 3 + 1) * 512],
                                 wsrc[:, :], wjunk[:, :],
                                 start=(i < 3), stop=(i >= 29))
            wp = ptp.tile([128, 512 * RW], F16, tag="p1", name="warmp")
            nc.scalar.activation(
                wp[:, :512], warm[:, :512],
                mybir.ActivationFunctionType.Exp, scale=ESC, bias=bias_t[:, :])

            qh_sb, k_sb, vh_sb, vl_sb = [], [], [], []
            for s in range(NSEG):
                qh = inp.tile([128, SEG], F16, tag=f"qh{s}", name=f"qh{s}")
                kk = inp.tile([128, SEG], F16, tag=f"k{s}", name=f"k{s}")
                vh = inp.tile([128, NKB * 65], F16, tag=f"vh{s}", name=f"vh{s}")
                vl = inp.tile([128, NKB * 65], F16, tag=f"vl{s}", name=f"vl{s}")
                vsl = slice(s * NKB * 65, (s + 1) * NKB * 65)
                # split the first segment's Q/K transfers across DMA queues so
                # round 0 isn't gated on a single ~512KB queue transfer
                nsl_dma = 4 if s == 0 else 1
                for t, ap_ in ((qh, qhh_ap), (kk, khl_ap)):
                    step = SEG // nsl_dma
                    for z in range(nsl_dma):
                        lo = z * step
                        nc.sync.dma_start(
                            t[:, lo:lo + step],
                            ap_[:, s * SEG + lo:s * SEG + lo + step])
                nc.sync.dma_start(vh[:, :], v1h_ap[:, vsl])
                nc.sync.dma_start(vl[:, :], v1l_ap[:, vsl])
                qh_sb.append(qh)
                k_sb.append(kk)
                vh_sb.append(vh)
                vl_sb.append(vl)

            ot_tiles = {}
            pend1, pend2 = [], []  # PV work lagged by 1 and 2 rounds

            def flush(items):
                for p1ref, i, u in items:
                    cid, kb = divmod(u, NKB)
                    s = cid // 4
                    if kb == 0:
                        ot_tiles[cid] = otp.tile([65, 512], F32, tag="ot",
                                                 name=f"ot{cid}")
                    vsl = slice(kb * 65, (kb + 1) * 65)
                    psl = slice(i * 512, (i + 1) * 512)
                    ot = ot_tiles[cid][:, :]
                    nc.tensor.matmul(ot, vh_sb[s][:, vsl], p1ref[:, psl],
                                     start=(kb == 0), stop=False)
                    nc.tensor.matmul(ot, vl_sb[s][:, vsl], p1ref[:, psl],
                                     start=False, stop=(kb == NKB - 1))
                    if kb == NKB - 1:
                        o_sb = osbp.tile([65, 512], F32, tag="osb",
                                         name=f"osb{cid}")
                        nc.vector.tensor_copy(o_sb[:, :], ot_tiles[cid][:, :])
                        nc.sync.dma_start(
                            out_ap[:, cid * 512:(cid + 1) * 512], o_sb[:, :])

            for r in range((NUNIT + RW - 1) // RW):
                units = range(r * RW, min((r + 1) * RW, NUNIT))
                nu = len(units)
                score = scp.tile([128, 512 * RW], F32, tag="score",
                                 name=f"score{r}")
                for i, u in enumerate(units):
                    cid, kb = divmod(u, NKB)
                    s, c = divmod(cid, 4)
                    osl = slice(i * 512, (i + 1) * 512)
                    csl = slice(c * 512, (c + 1) * 512)
                    lhsT = k_sb[s][:, kb * 128:(kb + 1) * 128]
                    nc.tensor.matmul(score[:, osl], lhsT, qh_sb[s][:, csl],
                                     start=True, stop=True)
                nsl = slice(0, 512 * nu)
                p1 = ptp.tile([128, 512 * RW], F16, tag="p1", name=f"p1_{r}")
                nc.scalar.activation(
                    p1[:, nsl], score[:, nsl],
                    mybir.ActivationFunctionType.Exp, scale=ESC,
                    bias=bias_t[:, :])
                if r < 2:
                    # startup filler: the first PV work arrives only after the
                    # round-0 scores->exp chain; keep the PE streaming through
                    # the pipe-fill with dummies aimed at an OT-pool slot
                    # (idle until round 2).
                    fill = otp.tile([65, 512], F32, tag="ot", name=f"fill{r}")
                    for z in range(6):
                        nc.tensor.matmul(fill[:, :], wsrc[:, :65], wjunk[:, :],
                                         start=(z == 0), stop=(z == 5))
                flush(pend2)
                pend2 = pend1
                pend1 = [(p1, i, u) for i, u in enumerate(units)]
            flush(pend2)
            flush(pend1)

    nc.compile()
    return nc


def _prep_core(query, key, value, core):
    b, j = divmod(core, 4)
    segs = []
    for arr in (query, key, value):
        h0 = arr[b, :, j, :].reshape(4, SEG, D)
        h1 = arr[b, :, 4 + j, :].reshape(2, 4096, D)[:, 1::2, :]
        h2 = arr[b, 2::4, 8 + j, :][None]
        segs.append(np.concatenate([h0, h1, h2], axis=0))  # [7, 2048, 64]
    qs, ks, vs = segs
    # [64, NSEG*SEG] with col = s*SEG + p
    qt = (qs * QSC).transpose(2, 0, 1).reshape(D, NSEG * SEG)
    kt = (ks * QSC).transpose(2, 0, 1).reshape(D, NSEG * SEG)
    qh = qt.astype(np.float16)
    kh = kt.astype(np.float16)
    kl = (kt - kh).astype(np.float16)
    vv = np.concatenate(
        [vs * QSC, np.full((NSEG, SEG, 1), 256.0, np.float32)],
        axis=2)  # [7, 2048, 65], pre-scaled
    v1 = vv.reshape(NSEG, NKB, 128, 65).transpose(2, 0, 1, 3).reshape(128, -1)
    v1h = v1.astype(np.float16)
    v1l = (v1 - v1h).astype(np.float16)
    return {
        "qhh": np.ascontiguousarray(np.concatenate([qh, qh], axis=0)),
        "khl": np.ascontiguousarray(np.concatenate([kh, kl], axis=0)),
        "v1h": np.ascontiguousarray(v1h),
        "v1l": np.ascontiguousarray(v1l),
    }


def _unshard(results, dtype):
    full = np.zeros((B, N, H, D), dtype)
    for core in range(8):
        b, j = divmod(core, 4)
        o = results[core]["out"].astype(np.float64)
        T = o[:64] / o[64:65]  # [64, 14336]
        h0 = T[:, :4 * SEG]
        full[b, :, j, :] = (h0 / (3.0 * h0.sum(1, keepdims=True))).T
        h1 = T[:, 4 * SEG:6 * SEG]
        h1 = h1 / (3.0 * h1.sum(1, keepdims=True))
        for g in range(2):
            full[b, g * 4096 + 1:(g + 1) * 4096:2, 4 + j, :] = \
                h1[:, g * SEG:(g + 1) * SEG].T
        h2 = T[:, 6 * SEG:]
        full[b, 2::4, 8 + j, :] = (h2 / (3.0 * h2.sum(1, keepdims=True))).T
    return full


def _ensure_axon_backend():
    """The bass PJRT path needs the axon/neuron jax backend. A harness may
    pin JAX_PLATFORMS=cpu for its reference; re-select axon if so."""
    import jax
    try:
        plat = jax.devices()[0].platform
    except Exception:
        plat = ""
    if plat not in ("axon", "neuron"):
        try:
            jax.config.update("jax_platforms", "axon,cpu")
            jax.devices()
        except Exception:
            pass


def kernel(query, key, value):
    _ensure_axon_backend()
    query = np.asarray(query, np.float32)
    key = np.asarray(key, np.float32)
    value = np.asarray(value, np.float32)
    assert query.shape == (B, N, H, D)

    if "nc" not in _CACHE:
        _CACHE["nc"] = _build_nc()
    nc = _CACHE["nc"]

    in_maps = [_prep_core(query, key, value, c) for c in range(8)]
    res = run_bass_kernel_spmd(nc, in_maps, core_ids=list(range(8)))
    LAST_RESULT["exec_time_ns"] = res.exec_time_ns
    return _unshard(res.results, query.dtype)
